# revision 1
# baseline (speedup 1.0000x reference)
"""KDE2D Trainium2 Bass kernel.

Reference computation (per (b,t) pair, B=16, T=64, N=512, grid 128x128):
  standardize points (mean/std ddof=1 over N), then
  density[gx,gy] = norm * sum_n exp(-c*(xg[gx]-x_n)^2) * exp(-c*(yg[gy]-y_n)^2)
  with c = 1/(2 h^2), norm = 1/(2 pi h^2).

Kernel strategy (data-parallel over the 1024 (b,t) pairs, 128 per core):
  exp(-c(g - x)^2) = [e^{-c g^2 + K}] * exp(2c*x*g - c*x^2 - K)
  The second factor is built per (bt, n-chunk) tile [n=128 part, g=128 free]
  with ONE ScalarE activation op: Exp(scale_p * GXROW + bias_p), where
  scale_p = 2c*x_p and bias_p = -c*x_p^2 - K are per-partition operands.
  bf16 tiles feed a 4-chunk accumulating PE matmul (contract n=512) into
  PSUM; the rank-1 factor beta_gx*beta_gy (which also carries norm and
  e^{2K}) is applied by one DVE scalar_tensor_tensor op, then DMA out.
  K keeps bf16/psum values in range (max product exponent 2*c*25 - 2K).
"""

import math

import numpy as np

import concourse.bass as bass
import concourse.bacc as bacc
import concourse.mybir as mybir
from concourse import tile
from concourse.bass_utils import run_bass_kernel_spmd

B, T, N, GRID = 16, 64, 512, 128
NCORES = 8
BT_PER_CORE = (B * T) // NCORES  # 128
NCHUNK = N // 128  # 4

F32 = mybir.dt.float32
BF16 = mybir.dt.bfloat16

_CACHE = {}


def _build(bw: float):
    h = float(bw)
    c = 1.0 / (2.0 * h * h)
    norm = 1.0 / (2.0 * math.pi * h * h)
    gmax = 5.0
    K = c * gmax * gmax / 2.0  # per-side exponent shift

    nc = bacc.Bacc("TRN2", target_bir_lowering=False)
    a_ext = nc.declare_dram_parameter("a", [BT_PER_CORE, N, 2], F32, isOutput=False)
    gx_ext = nc.declare_dram_parameter("gxrow", [128, GRID], F32, isOutput=False)
    idt_ext = nc.declare_dram_parameter("idt", [128, 128], F32, isOutput=False)
    bx_ext = nc.declare_dram_parameter("betax", [128, 1], F32, isOutput=False)
    by_ext = nc.declare_dram_parameter("betay", [128, GRID], F32, isOutput=False)
    out_ext = nc.declare_dram_parameter(
        "out", [BT_PER_CORE, GRID, GRID], F32, isOutput=True
    )

    AT = mybir.ActivationFunctionType
    OP = mybir.AluOpType

    with tile.TileContext(nc) as tc:
        with (
            tc.tile_pool(name="const", bufs=1) as cpool,
            tc.tile_pool(name="stats", bufs=1) as spool,
            tc.tile_pool(name="work", bufs=3) as wpool,
            tc.tile_pool(name="exy", bufs=12) as epool,
            tc.tile_pool(name="psum", bufs=6, space="PSUM") as ppool,
            tc.tile_pool(name="psumT", bufs=2, space="PSUM") as tpool,
            tc.tile_pool(name="outp", bufs=6) as opool,
        ):
            gx_sb = cpool.tile([128, GRID], F32, tag="gx")
            idt_sb = cpool.tile([128, 128], F32, tag="idt")
            bx_sb = cpool.tile([128, 1], F32, tag="bx")
            by_sb = cpool.tile([128, GRID], F32, tag="by")
            nc.sync.dma_start(gx_sb[:], gx_ext[:])
            nc.sync.dma_start(idt_sb[:], idt_ext[:])
            nc.sync.dma_start(bx_sb[:], bx_ext[:])
            nc.sync.dma_start(by_sb[:], by_ext[:])

            # ---- load points contiguously: [bt(128 part), n, ch] ----
            a_all = spool.tile([128, N, 2], F32, tag="a")
            nc.sync.dma_start(a_all[:], a_ext[:])
            x_sb = a_all[:, :, 0]
            y_sb = a_all[:, :, 1]

            # ---- per-bt stats and derived scale/bias arrays (layout [bt, n]) ----
            # sx = 2c * (x-mean)*invsd ; biasx = -c*((x-mean)*invsd)^2 - K
            derived = {}
            for ch, src in (("x", x_sb), ("y", y_sb)):
                s1 = spool.tile([128, 1], F32, tag=f"s1{ch}")
                s2 = spool.tile([128, 1], F32, tag=f"s2{ch}")
                sq = wpool.tile([128, N], F32, tag="sq")
                nc.vector.tensor_reduce(s1[:], src, mybir.AxisListType.X, OP.add)
                nc.vector.tensor_tensor(sq[:], src, src, OP.mult)
                nc.vector.tensor_reduce(s2[:], sq[:], mybir.AxisListType.X, OP.add)
                mean = spool.tile([128, 1], F32, tag=f"mean{ch}")
                nc.vector.tensor_scalar_mul(mean[:], s1[:], 1.0 / N)
                m2 = spool.tile([128, 1], F32, tag=f"m2{ch}")
                nc.vector.tensor_tensor(m2[:], mean[:], mean[:], OP.mult)
                var = spool.tile([128, 1], F32, tag=f"var{ch}")
                # var = (s2 - N*m2) / (N-1)
                nc.vector.scalar_tensor_tensor(
                    var[:], m2[:], -float(N), s2[:], OP.mult, OP.add
                )
                nc.vector.tensor_scalar_mul(var[:], var[:], 1.0 / (N - 1))
                sd = spool.tile([128, 1], F32, tag=f"sd{ch}")
                nc.scalar.activation(sd[:], var[:], AT.Sqrt)
                invsd = spool.tile([128, 1], F32, tag=f"invsd{ch}")
                nc.vector.reciprocal(invsd[:], sd[:])

                # xt = (x - mean) * invsd  (two tensor_scalar ops)
                xt = wpool.tile([128, N], F32, tag=f"xt{ch}")
                nc.vector.tensor_scalar(
                    xt[:], src, mean[:, 0:1], None, OP.subtract
                )
                nc.vector.tensor_scalar(
                    xt[:], xt[:], invsd[:, 0:1], None, OP.mult
                )
                # scale array: 2c * xt
                sc = wpool.tile([128, N], F32, tag=f"sc{ch}")
                nc.vector.tensor_scalar_mul(sc[:], xt[:], 2.0 * c)
                # bias array: -c*xt^2 - K
                bi = wpool.tile([128, N], F32, tag=f"bi{ch}")
                nc.vector.tensor_tensor(bi[:], xt[:], xt[:], OP.mult)
                nc.vector.tensor_scalar(bi[:], bi[:], -c, -K, OP.mult, OP.add)
                derived[ch] = (sc, bi)

            # ---- transpose derived arrays to [n(part), bt] via PE ----
            # Matmult instructions only tolerate ONE sync wait in walrus
            # codegen, so absorb the idt/gx DMA ticks into PE/ACT clocks
            # with dummy ops before the real transposes run.
            dummy_pt = tpool.tile([128, 128], F32, tag="pt")
            nc.tensor.transpose(dummy_pt[:], idt_sb[:], idt_sb[:])
            gx_probe = spool.tile([128, 1], F32, tag="gxprobe")
            nc.scalar.activation(gx_probe[:], gx_sb[:, 0:1], AT.Copy)
            # arrT[cc][:, bt] columns feed activation scale/bias operands.
            trans = {}
            for name, arr in (
                ("scx", derived["x"][0]),
                ("bix", derived["x"][1]),
                ("scy", derived["y"][0]),
                ("biy", derived["y"][1]),
            ):
                tiles = []
                for cc in range(NCHUNK):
                    pt = tpool.tile([128, 128], F32, tag="pt")
                    nc.tensor.transpose(
                        pt[:], arr[:, cc * 128 : (cc + 1) * 128], idt_sb[:]
                    )
                    st = cpool.tile([128, 128], F32, tag=f"T{name}{cc}")
                    nc.vector.tensor_copy(st[:], pt[:])
                    tiles.append(st)
                trans[name] = tiles

            # ---- main loop: one (bt) per iteration ----
            for bt in range(BT_PER_CORE):
                ps = ppool.tile([128, GRID], F32, tag="ps")
                exs, eys = [], []
                for cc in range(NCHUNK):
                    ex = epool.tile([128, GRID], BF16, tag="ex")
                    ey = epool.tile([128, GRID], BF16, tag="ey")
                    nc.scalar.activation(
                        ex[:], gx_sb[:], AT.Exp,
                        bias=trans["bix"][cc][:, bt : bt + 1],
                        scale=trans["scx"][cc][:, bt : bt + 1],
                    )
                    nc.scalar.activation(
                        ey[:], gx_sb[:], AT.Exp,
                        bias=trans["biy"][cc][:, bt : bt + 1],
                        scale=trans["scy"][cc][:, bt : bt + 1],
                    )
                    exs.append(ex)
                    eys.append(ey)
                for cc in range(NCHUNK):
                    nc.tensor.matmul(
                        ps[:], exs[cc][:], eys[cc][:],
                        start=(cc == 0), stop=(cc == NCHUNK - 1),
                    )
                ob = opool.tile([128, GRID], F32, tag="ob")
                # out = (psum * betax_p) * betay_row  (one DVE op)
                nc.vector.scalar_tensor_tensor(
                    ob[:], ps[:], bx_sb[:, 0:1], by_sb[:], OP.mult, OP.mult
                )
                nc.sync.dma_start(out_ext[bt], ob[:])

    if not nc.is_finalized():
        nc.finalize()
    return nc


def _consts(bw: float):
    h = float(bw)
    c = 1.0 / (2.0 * h * h)
    norm = 1.0 / (2.0 * math.pi * h * h)
    gmax = 5.0
    K = c * gmax * gmax / 2.0
    xg = np.linspace(-5.0, 5.0, GRID, dtype=np.float64)
    gxrow = np.broadcast_to(xg.astype(np.float32), (128, GRID)).copy()
    idt = np.eye(128, dtype=np.float32)
    betax = np.exp(K - c * xg * xg).astype(np.float32).reshape(GRID, 1)
    betay = (norm * np.exp(K - c * xg * xg)).astype(np.float32)
    betay = np.broadcast_to(betay, (128, GRID)).copy()
    return gxrow, idt, betax, betay


def kernel(A: np.ndarray, bandwidth: np.ndarray) -> np.ndarray:
    A = np.asarray(A, dtype=np.float32)
    bw = float(np.asarray(bandwidth))
    key = round(bw, 9)
    if key not in _CACHE:
        _CACHE[key] = _build(bw)
    nc = _CACHE[key]

    gxrow, idt, betax, betay = _consts(bw)
    a_flat = A.reshape(B * T, N, 2)
    in_maps = []
    for i in range(NCORES):
        in_maps.append(
            {
                "a": np.ascontiguousarray(
                    a_flat[i * BT_PER_CORE : (i + 1) * BT_PER_CORE]
                ),
                "gxrow": gxrow,
                "idt": idt,
                "betax": betax,
                "betay": betay,
            }
        )
    res = run_bass_kernel_spmd(nc, in_maps, core_ids=list(range(NCORES)))
    outs = [res.results[i]["out"] for i in range(NCORES)]
    return np.concatenate(outs, axis=0).reshape(B, T, GRID, GRID)


if __name__ == "__main__":
    A = np.random.randn(B, T, N, 2).astype(np.float32)
    out = kernel(A, np.float32(0.5))
    print(out.shape, out.dtype, float(out.max()))



# revision 20
# speedup vs baseline: 3.9911x; 3.9911x over previous
"""KDE2D Trainium2 Bass kernel — coarse-grid + LS-upsample formulation.

Reference (per (b,t), B=16, T=64, N=512, grid 128x128):
  standardize points (mean/std ddof=1 over N), then
  density[gx,gy] = norm * sum_n exp(-c(gx-x_n)^2) exp(-c(gy-y_n)^2),
  c = 1/(2 h^2), norm = 1/(2 pi h^2).

Strategy (128 (b,t) pairs per core, data-parallel over 8 cores):
  The Gaussian factor exp(-c(g-x)^2) as a function of g is smooth on the
  bandwidth scale, so evaluate it on a coarse GC=20-point grid and lift the
  coarse density to the 128-point grid with a least-squares interpolation
  matrix U fitted offline over the translate family {exp(-c(g-x)^2)}_x:
      D = U (Exc^T Eyc) U^T,   U: [128, GC].

  exp on the coarse grid is computed with ONE ScalarE instruction per
  (coarse point, bt-half) in the transposed layout [n(part), (side,chunk,bt)]:
      EXY = Exp(2c*gc[g] * XYT - c*gc[g]^2)        (scale/bias are scalars!)
  which drops the per-(bt,chunk,side) activation count of the naive layout
  (1024 instrs -> 40), sidestepping ScalarE's 185ns/instr SBUF overhead.
  The missing per-point factor exp(-c(x^2+y^2)) is folded into the x-side
  by one DVE multiply per coarse point (RONE = [R | ones]); folding both
  matmul operands through DVE keeps every matmul at <=1 semaphore wait.

  Per bt: Dct[j,i] = sum_n ey'[n,j] ex''[n,i]   (PE, 4 n-chunks, bf16)
          T1[(bt,i), gy] = Dct^T U^T             (PE, 4 bt batched)
          D[gx, gy] = U^T-weighted lift of T1    (PE)
  PSUM->SBUF copies are spread over Pool/DVE/ACT; output DMA is batched
  4 bt per transfer (HWDGE issue overhead is ~625ns serialized).
"""

import math
import os

import numpy as np

import concourse.bass as bass
import concourse.bacc as bacc
import concourse.mybir as mybir
from concourse import tile
from concourse.bass_utils import run_bass_kernel_spmd

B, T, N, GRID = 16, 64, 512, 128
NCORES = 8
BT_PER_CORE = (B * T) // NCORES  # 128
NCH = N // 128  # 4
GC = 20  # coarse grid points
NH = 2  # bt halves per core
BTH = BT_PER_CORE // NH  # 64

F32 = mybir.dt.float32
BF16 = mybir.dt.bfloat16

_CACHE = {}


def _build(bw: float):
    STAGE = int(os.environ.get("STAGE", "4"))
    h = float(bw)
    c = 1.0 / (2.0 * h * h)
    gcg = np.linspace(-5.0, 5.0, GC, dtype=np.float64)

    nc = bacc.Bacc("TRN2", target_bir_lowering=False)
    a_ext = nc.declare_dram_parameter("a", [BT_PER_CORE, N, 2], F32, isOutput=False)
    idt_ext = nc.declare_dram_parameter("idt", [128, 128], F32, isOutput=False)
    ut_ext = nc.declare_dram_parameter("ut", [128, GRID], F32, isOutput=False)
    sb_ext = nc.declare_dram_parameter("sb", [128, 2 * GC], F32, isOutput=False)
    # out layout (gx, bt, gy) so a 4-bt batch is one contiguous-dst DMA
    out_ext = nc.declare_dram_parameter(
        "out", [GRID, BT_PER_CORE, GRID], F32, isOutput=True
    )

    AT = mybir.ActivationFunctionType
    OP = mybir.AluOpType

    with tile.TileContext(nc) as tc:
        with (
            tc.tile_pool(name="const", bufs=1) as cpool,
            tc.tile_pool(name="stats", bufs=1) as spool,
            tc.tile_pool(name="big", bufs=1) as bpool,
            tc.tile_pool(name="dcts", bufs=1) as dpool,
            tc.tile_pool(name="t14s", bufs=4) as t14pool,
            tc.tile_pool(name="stage", bufs=6) as stpool,
            tc.tile_pool(name="pdct", bufs=2, space="PSUM") as dctpool,
            tc.tile_pool(name="pt14", bufs=2, space="PSUM") as t14ppool,
        ):
            idt_sb = cpool.tile([128, 128], F32, tag="idt")
            ut_f = cpool.tile([128, GRID], F32, tag="utf")
            ut_bf = cpool.tile([128, GRID], BF16, tag="utb")
            sb_sb = cpool.tile([128, 2 * GC], F32, tag="sb")
            nc.sync.dma_start(idt_sb[:], idt_ext[:])
            nc.sync.dma_start(ut_f[:], ut_ext[:])
            nc.sync.dma_start(sb_sb[:], sb_ext[:])

            a_all = spool.tile([128, N, 2], F32, tag="a")
            nc.sync.dma_start(a_all[:], a_ext[:])

            with tc.tile_pool(name="ptrans", bufs=2, space="PSUM") as trpool:
                # bf16 U^T (DVE) then PE dummies absorb the const DMA sems
                nc.vector.tensor_copy(ut_bf[:], ut_f[:])
                trA = trpool.tile([128, 512], F32, tag="tr")
                nc.tensor.transpose(
                    trA[:, 0:64], idt_sb[0:64, :], idt_sb[0:64, 0:64]
                )
                nc.tensor.matmul(
                    trA[:, 64:128], ut_bf[0:GC, :], ut_bf[0:GC, 0:64],
                    start=True, stop=True,
                )

                # ---- per-bt stats (ddof=1) + standardization, layout [bt,n] ----
                std = {}
                for ch, idx in (("x", 0), ("y", 1)):
                    src = a_all[:, :, idx]
                    s1 = spool.tile([128, 1], F32, tag=f"s1{ch}")
                    sq = spool.tile([128, N], F32, tag=f"sq{ch}")
                    s2 = spool.tile([128, 1], F32, tag=f"s2{ch}")
                    nc.vector.tensor_reduce(s1[:], src, mybir.AxisListType.X, OP.add)
                    nc.vector.tensor_tensor(sq[:], src, src, OP.mult)
                    nc.vector.tensor_reduce(s2[:], sq[:], mybir.AxisListType.X, OP.add)
                    mean = spool.tile([128, 1], F32, tag=f"mean{ch}")
                    nc.vector.tensor_scalar_mul(mean[:], s1[:], 1.0 / N)
                    m2 = spool.tile([128, 1], F32, tag=f"m2{ch}")
                    nc.vector.tensor_tensor(m2[:], mean[:], mean[:], OP.mult)
                    var = spool.tile([128, 1], F32, tag=f"var{ch}")
                    nc.vector.scalar_tensor_tensor(
                        var[:], m2[:], -float(N), s2[:], OP.mult, OP.add
                    )
                    nc.vector.tensor_scalar_mul(var[:], var[:], 1.0 / (N - 1))
                    sd = spool.tile([128, 1], F32, tag=f"sd{ch}")
                    nc.scalar.activation(sd[:], var[:], AT.Sqrt)
                    invsd = spool.tile([128, 1], F32, tag=f"invsd{ch}")
                    nc.vector.reciprocal(invsd[:], sd[:])
                    xt = spool.tile([128, N], F32, tag=f"xt{ch}")
                    nc.vector.tensor_scalar(
                        xt[:], src, mean[:, 0:1], None, OP.subtract
                    )
                    nc.vector.tensor_scalar(
                        xt[:], xt[:], invsd[:, 0:1], None, OP.mult
                    )
                    std[ch] = xt

                # ---- r = exp(-c(x^2+y^2)) in [bt, n] layout ----
                t1 = spool.tile([128, N], F32, tag="t1")
                t2 = spool.tile([128, N], F32, tag="t2")
                r2 = spool.tile([128, N], F32, tag="r2")
                nc.vector.tensor_tensor(t1[:], std["x"][:], std["x"][:], OP.mult)
                nc.vector.tensor_tensor(t2[:], std["y"][:], std["y"][:], OP.mult)
                nc.vector.tensor_tensor(r2[:], t1[:], t2[:], OP.add)
                rb = spool.tile([128, N], F32, tag="rb")
                nc.scalar.activation(rb[:], r2[:], AT.Exp, scale=-c)

                # ---- transposes to [n(part), (side, chunk, bt-in-half)] ----
                # r first (fresh psum cols wait only on ACT), then x/y whose
                # WAR+input deps are both DVE -> one wait per transpose.
                rone = []
                xyt = []
                slot = 2  # cols 0:128 of trA hold the dummies
                cur = trA
                def tslot():
                    nonlocal slot, cur
                    if slot == 8:
                        cur = trpool.tile([128, 512], F32, tag="tr")
                        slot = 0
                    s = cur[:, slot * 64 : (slot + 1) * 64]
                    slot += 1
                    return s
                for hh in range(NH):
                    ro = bpool.tile([128, 2 * NCH * BTH], BF16, tag=f"rone{hh}")
                    rone.append(ro)
                    ib = idt_sb[hh * BTH : (hh + 1) * BTH, hh * BTH : (hh + 1) * BTH]
                    for cc in range(NCH):
                        pt = tslot()
                        nc.tensor.transpose(
                            pt,
                            rb[hh * BTH : (hh + 1) * BTH, cc * 128 : (cc + 1) * 128],
                            ib,
                        )
                        nc.vector.tensor_copy(ro[:, cc * BTH : (cc + 1) * BTH], pt)
                    nc.vector.memset(ro[:, NCH * BTH : 2 * NCH * BTH], 1.0)
                for hh in range(NH):
                    xy = bpool.tile([128, 2 * NCH * BTH], F32, tag=f"xyt{hh}")
                    xyt.append(xy)
                    ib = idt_sb[hh * BTH : (hh + 1) * BTH, hh * BTH : (hh + 1) * BTH]
                    for side, ch in ((0, "x"), (1, "y")):
                        for cc in range(NCH):
                            pt = tslot()
                            nc.tensor.transpose(
                                pt,
                                std[ch][
                                    hh * BTH : (hh + 1) * BTH,
                                    cc * 128 : (cc + 1) * 128,
                                ],
                                ib,
                            )
                            nc.vector.tensor_copy(
                                xy[:, side * NCH * BTH + cc * BTH :][:, 0:BTH], pt
                            )

            with tc.tile_pool(name="pout", bufs=4, space="PSUM") as outpool:
                # ---- main: exp on coarse grid, fold, contract, upsample ----
                W = 2 * NCH * BTH  # 512 free per g-slab
                copy_rr = 0
                for hh in range(NH):
                    exy = bpool.tile([128, GC * W], BF16, tag=f"exy{hh}")
                    exy2 = bpool.tile([128, GC * W], BF16, tag=f"exy2{hh}")
                    for g in range(GC):
                        nc.scalar.activation(
                            exy[:, g * W : (g + 1) * W],
                            xyt[hh][:],
                            AT.Exp,
                            bias=sb_sb[:, GC + g : GC + g + 1],
                            scale=sb_sb[:, g : g + 1],
                        )
                    feng = nc.vector
                    for g in range(GC):
                        feng.tensor_tensor(
                            exy2[:, g * W : (g + 1) * W],
                            exy[:, g * W : (g + 1) * W],
                            rone[hh][:],
                            OP.mult,
                        )
                    for pp in range(4):  # 16 bt per dct psum bank, rows 0:20
                        dct_p = dctpool.tile([128, 4, 4, 32], F32, tag="dct")
                        nc.tensor.matmul(
                            dct_p[:, :, :, :],
                            ut_bf[0:GC, :],
                            rone[hh][0:GC, :],
                            start=True,
                            stop=True,
                        )
                        for q4 in range(4):
                            for k in range(4):
                                btl = pp * 16 + q4 * 4 + k
                                for cc in range(NCH):
                                    yoff = NCH * BTH + cc * BTH + btl
                                    xoff = cc * BTH + btl
                                    nc.tensor.matmul(
                                        dct_p[0:GC, q4, k, 0:GC],
                                        exy2[:, yoff::W],
                                        exy2[:, xoff::W],
                                        start=(cc == 0),
                                        stop=(cc == NCH - 1),
                                    )
                        dcts = dpool.tile([128, 4, 4, 32], BF16, tag=f"dcts{hh}{pp}")
                        nc.vector.tensor_copy(dcts[0:GC, :, :, :], dct_p[0:GC, :, :, :])
                        for q4 in range(4):
                            t14_p = t14ppool.tile([128, 4, GRID], F32, tag="t14")
                            nc.tensor.matmul(
                                t14_p[:, :, :],
                                ut_bf[0:GC, :],
                                rone[hh][0:GC, :],
                                start=True,
                                stop=True,
                            )
                            for k in range(4):
                                nc.tensor.matmul(
                                    t14_p[0:32, k, :],
                                    dcts[0:GC, q4, k, :],
                                    ut_bf[0:GC, :],
                                    start=True,
                                    stop=True,
                                )
                            t14s = t14pool.tile([128, 4, GRID], BF16, tag="t14s")
                            teng = nc.vector if (q4 % 2) else nc.scalar
                            if teng is nc.scalar:
                                nc.scalar.copy(t14s[0:32, :, :], t14_p[0:32, :, :])
                            else:
                                teng.tensor_copy(t14s[0:32, :, :], t14_p[0:32, :, :])
                            out_p = outpool.tile([128, 4, GRID], F32, tag="outp")
                            # WAR-carrier: dummy PE write claims the bank so the
                            # real matmuls keep a single data-dep wait.
                            nc.tensor.matmul(
                                out_p[:, :, :],
                                ut_bf[0:GC, :],
                                rone[hh][0:GC, :],
                                start=True,
                                stop=True,
                            )
                            for k in range(4):
                                nc.tensor.matmul(
                                    out_p[:, k, :],
                                    ut_bf[0:GC, :],
                                    t14s[0:GC, k, :],
                                    start=True,
                                    stop=True,
                                )
                            stg = stpool.tile([128, 4, GRID], F32, tag="stage")
                            eng = (nc.vector, nc.scalar, nc.vector)[copy_rr % 3]
                            copy_rr += 1
                            if eng is nc.scalar:
                                nc.scalar.copy(stg[:], out_p[:])
                            else:
                                eng.tensor_copy(stg[:], out_p[:])
                            bt0 = hh * BTH + pp * 16 + q4 * 4
                            nc.sync.dma_start(out_ext[:, bt0 : bt0 + 4, :], stg[:])

    if not nc.is_finalized():
        nc.finalize()
    return nc


def _consts(bw: float):
    h = float(bw)
    c = 1.0 / (2.0 * h * h)
    norm = 1.0 / (2.0 * math.pi * h * h)
    gf = np.linspace(-5.0, 5.0, GRID, dtype=np.float64)
    gcg = np.linspace(-5.0, 5.0, GC, dtype=np.float64)
    xs = np.linspace(-9.0, 9.0, 6001)
    Ff = np.exp(-c * (gf[:, None] - xs) ** 2)
    Fc = np.exp(-c * (gcg[:, None] - xs) ** 2)
    U = np.linalg.lstsq(Fc.T, Ff.T, rcond=None)[0].T  # [GRID, GC]
    utr = (U * math.sqrt(norm)).T.astype(np.float32)  # [GC, GRID]
    ut = np.zeros((128, GRID), dtype=np.float32)
    for qb in (0, 32, 64):
        ut[qb : qb + GC] = utr  # replicate at each PE quadrant base
    idt = np.eye(128, dtype=np.float32)
    sb = np.concatenate([2.0 * c * gcg, -c * gcg * gcg]).astype(np.float32)
    sb = np.broadcast_to(sb, (128, 2 * GC)).copy()
    return idt, np.ascontiguousarray(ut), sb


def kernel(A: np.ndarray, bandwidth: np.ndarray) -> np.ndarray:
    A = np.asarray(A, dtype=np.float32)
    bw = float(np.asarray(bandwidth))
    key = round(bw, 9)
    if key not in _CACHE:
        _CACHE[key] = _build(bw)
    nc = _CACHE[key]

    idt, ut, sb = _consts(bw)
    a_flat = A.reshape(B * T, N, 2)
    in_maps = []
    for i in range(NCORES):
        in_maps.append(
            {
                "a": np.ascontiguousarray(
                    a_flat[i * BT_PER_CORE : (i + 1) * BT_PER_CORE]
                ),
                "idt": idt,
                "ut": ut,
                "sb": sb,
            }
        )
    res = run_bass_kernel_spmd(nc, in_maps, core_ids=list(range(NCORES)))
    outs = [
        res.results[i]["out"].transpose(1, 0, 2) for i in range(NCORES)
    ]  # (gx,bt,gy)->(bt,gx,gy)
    return np.concatenate(outs, axis=0).reshape(B, T, GRID, GRID)


if __name__ == "__main__":
    A = np.random.randn(B, T, N, 2).astype(np.float32)
    out = kernel(A, np.float32(0.5))
    print(out.shape, out.dtype, float(out.max()))
